# revision 51
# baseline (speedup 1.0000x reference)
"""Trainium2 Bass kernel for a GQA attention block (RMSNorm -> QKV+gate ->
Q/K-norm -> RoPE -> attention -> gated out -> proj), tensor-parallel over
heads across 8 NeuronCores.

Sharding: core c owns q heads [5c, 5c+5) and kv group c (NQ=40, NKV=8).
Each core computes a partial projection output; partials are summed on host
(row-parallel proj unshard).

v1: bf16 matmul operands everywhere (f32 PSUM accumulate, f32 softmax /
norm statistics), qkv weights and rope tables resident in SBUF.
"""
import sys

sys.path.insert(0, "/opt/trn_rl_repo")

import ml_dtypes
import numpy as np

import bass_rust as _bass_rust

import concourse.bacc as bacc
import concourse.tile as tile
from concourse import mybir
from concourse.hw_specs import get_activation_tables


class _Bacc(bacc.Bacc):
    """Bacc with activation-table choice restricted to the exp+ln set.

    The stock insert_act_table_loads pass picks the FIRST act_func_set
    containing each function, so alternating Ln/Exp activations thrash
    between two tables.  Emptying all sets except `natural_log_exp_and_others`
    (square/copy/exp/ln) and `sigmoid_and_others` forces one resident table
    for the whole kernel (plus a single swap around the sigmoid).
    """

    _KEEP_SETS = {"natural_log_exp_and_others", "sigmoid_and_others"}

    def insert_act_table_loads(self):
        has_activation = any(
            isinstance(i, mybir.InstActivation)
            for b in self.main_func.blocks
            for i in b.instructions
        )
        if not has_activation:
            return
        tables = [
            (name, (fns if name in self._KEEP_SETS else set()))
            for name, fns in get_activation_tables(self.m.arch).items()
        ]
        _bass_rust.insert_act_table_loads(self, tables)

NQ, NKV, D, HID = 40, 8, 128, 5120
S = 2048
NC = 8
HPC = NQ // NC          # q heads per core = 5
EPS = 1e-6
HT = HID // 128         # 40 hid tiles
ST = S // 128           # 16 seq tiles of 128
NCH = S // 512          # 4 chunks of 512
KT = S // 128           # 16 k-tiles
QKV_COLS = HPC * D + 2 * D + HPC   # 901
F32 = mybir.dt.float32
F32R = mybir.dt.float32r
BF16 = mybir.dt.bfloat16
AF = mybir.ActivationFunctionType
# build-time tunables (A/B-tested defaults are read inside build_program)
BUILD_OPTS = {}


def build_program(repeat=1):
    opt = BUILD_OPTS
    nc = _Bacc(None, target_bir_lowering=False)

    # register activation-bias constants (mirrors Bass.__init__ registration)
    for val in (EPS, float(D) * EPS):
        t = nc.alloc_sbuf_tensor(f"const-float32-{val}", [128, 1], F32)
        nc.gpsimd.memset(t.ap(), val)
        nc.const_aps.aps[(F32, val)] = t.ap()
    nc.all_engine_barrier()

    # ---- I/O ----
    xT = nc.dram_tensor("xT", [HT, 128, S], BF16, kind="ExternalInput")
    wq = nc.dram_tensor("wq", [HT, 128, QKV_COLS], BF16, kind="ExternalInput")
    wp = nc.dram_tensor("wp", [HPC, 128, HID], BF16, kind="ExternalInput")
    cosq = nc.dram_tensor("cosq", [128, S], BF16, kind="ExternalInput")
    sinq = nc.dram_tensor("sinq", [128, S], BF16, kind="ExternalInput")
    cosk = nc.dram_tensor("cosk", [128, S], BF16, kind="ExternalInput")
    sink = nc.dram_tensor("sink", [128, S], BF16, kind="ExternalInput")
    ones_col = nc.dram_tensor("ones_col", [128, 1], F32R, kind="ExternalInput")
    ident = nc.dram_tensor("ident", [128, 128], BF16, kind="ExternalInput")
    out = nc.dram_tensor("out", [S, HID], F32, kind="ExternalOutput")

    with tile.TileContext(nc, pool_alloc_mode=opt.get("palloc", "stack")) as tc:
      for _rep in range(repeat):
        with tc.tile_pool(name=f"persist{_rep}", bufs=1) as pers, \
             tc.tile_pool(name=f"cols{_rep}", bufs=1) as cols, \
             tc.tile_pool(name=f"scr{_rep}", bufs=1, space="DRAM") as dscr:
            # DRAM row bounces (for partition-broadcast / row->col reshape)
            lnm_scr = dscr.tile([1, S], F32, name="lnm_scr")
            lnk_scr = dscr.tile([1, S], F32, name="lnk_scr")
            rrow_scr = dscr.tile([1, S], F32, name="rrow_scr")
            gate_scr = dscr.tile([HPC, S], F32, name="gate_scr")
            nq_scr = dscr.tile([HPC, S], F32, name="nq_scr")
            scale_scr = dscr.tile([HPC, S], F32, name="scale_scr")
            # persistent small tiles
            t_ones = cols.tile([128, 1], F32R, name="ones")
            nc.sync.dma_start(t_ones[:, :], ones_col[:, :])
            t_id = cols.tile([128, 128], BF16, name="ident")
            nc.sync.dma_start(t_id[:, :], ident[:, :])
            t_ones_b = cols.tile([128, 1], BF16, name="ones_b")
            nc.gpsimd.tensor_copy(t_ones_b[:, :], t_ones[:, :])


            # resident qkv weights (bf16) and rope tables.  The weight loads
            # are interleaved with the first chunk's x loads inside phase 1
            # (same FIFO queue) so the PE isn't starved at startup; the rope
            # tables ride a different engine's queue.
            wq_sb = pers.tile([128, HT, QKV_COLS], BF16, name="wq_sb")
            t_cosq = pers.tile([128, S], BF16, name="cosq")
            t_sinq = pers.tile([128, S], BF16, name="sinq")
            t_cosk = pers.tile([128, S], BF16, name="cosk")
            t_sink = pers.tile([128, S], BF16, name="sink")
            nc.scalar.dma_start(t_cosq[:, :], cosq[:, :])
            nc.scalar.dma_start(t_sinq[:, :], sinq[:, :])
            nc.scalar.dma_start(t_cosk[:, :], cosk[:, :])
            nc.scalar.dma_start(t_sink[:, :], sink[:, :])

            q_t = [pers.tile([128, S], BF16, name=f"q{h}") for h in range(HPC)]
            k_t = pers.tile([128, S], BF16, name="kT")
            vnat = pers.tile([128, S], BF16, name="vnat")
            o_t = [pers.tile([128, S], BF16, name=f"o{h}") for h in range(HPC)]

            with tc.tile_pool(name=f"vg{_rep}", bufs=1) as vg:
                v_t = vg.tile([128, S], BF16, name="vT")
                gates = vg.tile([5, S], F32, name="gates")

                # ============ Phase 1: QKV matmul (+ x^2 accumulation) ======
                with tc.tile_pool(name=f"qkv_ps{_rep}", bufs=1, space="PSUM") as psA, \
                     tc.tile_pool(name=f"qkv_psB{_rep}", bufs=1, space="PSUM") as psB, \
                     tc.tile_pool(name=f"qkv_sb{_rep}", bufs=opt.get("qkv_bufs", 3)) as sb2, \
                     tc.tile_pool(name=f"qkv_sb3{_rep}", bufs=3) as sb3:
                    for ch in range(NCH):
                        c0 = ch * 512
                        pm = [psA.tile([128, 512], F32, name=f"mt{m}")
                              for m in range(7)]
                        pg = psB.tile([5, 512], F32, name="small")
                        acc = sb2.tile([128, 1024], F32, name="accsq")
                        accr = sb2.tile([128, 512], F32R, name="accr")
                        NP = HT // 2
                        for hp in range(NP):
                            ht = 2 * hp
                            if ch == 0:
                                nc.sync.dma_start(
                                    wq_sb[:, ht:ht + 2, :],
                                    wq[ht:ht + 2, :, :].rearrange(
                                        "t p c -> p t c"))
                            xt = sb2.tile([128, 2, 512], BF16, name="xt")
                            nc.sync.dma_start(
                                xt[:, :, :],
                                xT[ht:ht + 2, :, c0:c0 + 512].rearrange(
                                    "t p c -> p t c"))
                            for i in range(2):
                                for m in range(7):
                                    nc.tensor.matmul(
                                        pm[m][:, :],
                                        wq_sb[:, ht + i, m * 128:(m + 1) * 128],
                                        xt[:, i, :], start=(hp == 0 and i == 0),
                                        stop=(hp == NP - 1 and i == 1))
                                nc.tensor.matmul(pg[:, :],
                                                 wq_sb[:, ht + i, 896:901],
                                                 xt[:, i, :],
                                                 start=(hp == 0 and i == 0),
                                                 stop=(hp == NP - 1 and i == 1))
                            sq = sb3.tile([128, 1024], F32, name="sq")
                            nc.scalar.activation(
                                sq[:, :],
                                xt[:, :, :].rearrange("p t c -> p (t c)"),
                                AF.Square)
                            if hp == 0:
                                nc.gpsimd.tensor_copy(acc[:, :], sq[:, :])
                            else:
                                nc.gpsimd.tensor_add(acc[:, :], acc[:, :], sq[:, :])
                        nc.vector.tensor_add(accr[:, :], acc[:, 0:512],
                                             acc[:, 512:1024])
                        use_split = opt.get("drain_split", True)
                        for h in range(HPC):
                            if use_split and h in (1, 3):
                                nc.scalar.copy(q_t[h][:, c0:c0 + 512], pm[h][:, :])
                            else:
                                nc.vector.tensor_copy(q_t[h][:, c0:c0 + 512],
                                                      pm[h][:, :])
                        if use_split:
                            nc.scalar.copy(k_t[:, c0:c0 + 512], pm[5][:, :])
                        else:
                            nc.vector.tensor_copy(k_t[:, c0:c0 + 512], pm[5][:, :])
                        nc.vector.tensor_copy(v_t[:, c0:c0 + 512], pm[6][:, :])
                        nc.vector.tensor_copy(gates[:, c0:c0 + 512], pg[:, :])
                        pr = psB.tile([1, 512], F32, name="small")
                        nc.tensor.matmul(pr[:, :], t_ones[:, :], accr[:, :],
                                         start=True, stop=True)
                        lnm_row = sb2.tile([1, 512], F32, name="lnm_row")
                        nc.scalar.activation(lnm_row[:, :], pr[:, :], AF.Ln,
                                             bias=EPS, scale=1.0 / HID)
                        nc.sync.dma_start(lnm_scr[0:1, c0:c0 + 512], lnm_row[:, :])
                        r_row = sb2.tile([1, 512], F32, name="r_row")
                        nc.scalar.activation(r_row[:, :], lnm_row[:, :], AF.Exp,
                                             bias=0.0, scale=-0.5)
                        nc.sync.dma_start(rrow_scr[0:1, c0:c0 + 512], r_row[:, :])

                # ============ Phase 2a: v transpose (x r) + gates ===========
                with tc.tile_pool(name=f"tr_ps{_rep}", bufs=2, space="PSUM") as pTr, \
                     tc.tile_pool(name=f"ga_sb{_rep}", bufs=2) as gsb:
                    lnm_col = cols.tile([128, KT], F32, name="lnm_col")
                    nc.sync.dma_start(lnm_col[:, :],
                                      lnm_scr[0, :].rearrange("(t p) -> p t", p=128))
                    r_col = cols.tile([128, KT], F32, name="r_col")
                    nc.scalar.activation(r_col[:, :], lnm_col[:, :], AF.Exp,
                                         bias=0.0, scale=-0.5)
                    for kt in range(KT):
                        ptr = pTr.tile([128, 128], BF16, name="tr")
                        nc.tensor.transpose(ptr[:, :], v_t[:, kt * 128:(kt + 1) * 128],
                                            t_id[:, :])
                        nc.vector.tensor_scalar_mul(vnat[:, kt * 128:(kt + 1) * 128],
                                                    ptr[:, :], r_col[:, kt:kt + 1])
                    for ch in range(NCH):
                        c0 = ch * 512
                        rb = gsb.tile([5, 512], F32, name="rhatb")
                        nc.sync.dma_start(
                            rb[:, :],
                            rrow_scr[0:1, c0:c0 + 512].to_broadcast((5, 512)))
                        nc.vector.tensor_mul(gates[:, c0:c0 + 512],
                                             gates[:, c0:c0 + 512], rb[:, :])
                    nc.scalar.activation(gates[:, :], gates[:, :], AF.Sigmoid)
                    nc.sync.dma_start(gate_scr[:, :], gates[:, :])
                    # ---- k: norm sums + rope here so they overlap the gate
                    # and v work instead of serializing at attention start
                    for ch in range(NCH):
                        c0 = ch * 512
                        ksq = gsb.tile([128, 512], BF16, name="ksqc")
                        nc.scalar.activation(ksq[:, :], k_t[:, c0:c0 + 512],
                                             AF.Square)
                        pn = pTr.tile([1, 512], F32, name="prow")
                        nc.tensor.matmul(pn[:, :], t_ones_b[:, :], ksq[:, :],
                                         start=True, stop=True)
                        lnk_row = gsb.tile([1, 512], F32, name="lnkrow")
                        nc.scalar.activation(lnk_row[:, :], pn[:, :], AF.Ln,
                                             bias=D * EPS, scale=1.0)
                        nc.sync.dma_start(lnk_scr[0:1, c0:c0 + 512],
                                          lnk_row[:, :])
                    for c2 in range(2):
                        c0 = c2 * 1024
                        rot = gsb.tile([128, 1024], BF16, name="rotk")
                        nc.sync.dma_start(rot[0:64, :], k_t[64:128, c0:c0 + 1024])
                        nc.sync.dma_start(rot[64:128, :], k_t[0:64, c0:c0 + 1024])
                        t1 = gsb.tile([128, 1024], F32, name="t1k")
                        rr = gsb.tile([128, 1024], F32, name="rrk")
                        nc.vector.tensor_mul(t1[:, :], k_t[:, c0:c0 + 1024],
                                             t_cosk[:, c0:c0 + 1024])
                        nc.vector.tensor_mul(rr[:, :], rot[:, :],
                                             t_sink[:, c0:c0 + 1024])
                        nc.vector.tensor_add(k_t[:, c0:c0 + 1024], t1[:, :],
                                             rr[:, :])
            # vg pool (v_t, gates) closes here

            # ============ Phase 2b/3: norms, rope, attention ================
            with tc.tile_pool(name=f"at_row{_rep}", bufs=2, space="PSUM") as pRow, \
                 tc.tile_pool(name=f"at_sc{_rep}", bufs=2, space="PSUM") as pSC, \
                 tc.tile_pool(name=f"at_av{_rep}", bufs=1, space="PSUM") as pAV, \
                 tc.tile_pool(name=f"at_sb1{_rep}", bufs=opt.get("acc_bufs", 1)) as asb1, \
                 tc.tile_pool(name=f"at_sb2{_rep}", bufs=2) as asb2, \
                 tc.tile_pool(name=f"at_rp{_rep}", bufs=1) as rsb, \
                 tc.tile_pool(name=f"at_sb3{_rep}", bufs=opt.get("expt_bufs", 3)) as asb3:

                lnk_col = cols.tile([128, KT], F32, name="lnk_col")
                nc.sync.dma_start(lnk_col[:, :],
                                  lnk_scr[0, :].rearrange("(t p) -> p t", p=128))
                nk_col = cols.tile([128, KT], F32, name="nk_col")
                nc.scalar.activation(nk_col[:, :], lnk_col[:, :], AF.Exp,
                                     bias=0.0, scale=-0.5)

                # ---- per head: norm + rope, then attention, software-
                # pipelined: rope(h+1) is emitted before attention(h) so the
                # in-order DVE stream ropes the next head while the PE works
                # on the current head's attention.
                def emit_rope(h):
                    for c2 in range(2):
                        c0 = c2 * 1024
                        for j in range(2):
                            s0 = c0 + j * 512
                            qsq = asb2.tile([128, 512], BF16, name="sqc")
                            # square on DVE (mul) — ACT is the bottleneck in
                            # the attention phase
                            nc.vector.tensor_mul(qsq[:, :], q_t[h][:, s0:s0 + 512],
                                                 q_t[h][:, s0:s0 + 512])
                            pn = pRow.tile([1, 512], F32, name="row")
                            nc.tensor.matmul(pn[:, :], t_ones_b[:, :], qsq[:, :],
                                             start=True, stop=True)
                            lnq = asb2.tile([1, 512], F32, name="lnq")
                            nc.scalar.activation(lnq[:, :], pn[:, :], AF.Ln,
                                                 bias=EPS, scale=1.0 / D)
                            nqr = asb2.tile([1, 512], F32, name="nqrow")
                            nc.scalar.activation(nqr[:, :], lnq[:, :], AF.Exp,
                                                 bias=0.0, scale=-0.5)
                            nc.sync.dma_start(nq_scr[h:h + 1, s0:s0 + 512], nqr[:, :])
                        nb = asb2.tile([128, 1024], F32, name="nb")
                        nc.sync.dma_start(
                            nb[:, :],
                            nq_scr[h:h + 1, c0:c0 + 1024].to_broadcast((128, 1024)))
                        rot = asb2.tile([128, 1024], BF16, name="rot")
                        nc.sync.dma_start(rot[0:64, :], q_t[h][64:128, c0:c0 + 1024])
                        nc.sync.dma_start(rot[64:128, :], q_t[h][0:64, c0:c0 + 1024])
                        t1 = rsb.tile([128, 1024], F32, name="t1")
                        rr = rsb.tile([128, 1024], F32, name="rr")
                        nc.vector.tensor_mul(t1[:, :], q_t[h][:, c0:c0 + 1024],
                                             t_cosq[:, c0:c0 + 1024])
                        nc.vector.tensor_mul(rr[:, :], rot[:, :],
                                             t_sinq[:, c0:c0 + 1024])
                        nc.vector.tensor_add(t1[:, :], t1[:, :], rr[:, :])
                        nc.vector.tensor_mul(q_t[h][:, c0:c0 + 1024], t1[:, :],
                                             nb[:, :])

                def emit_attn(h):
                    for qp in range(2):
                        c0 = qp * 1024
                        po = [pAV.tile([128, 512], F32, name=f"av{j}")
                              for j in range(2)]
                        accA = asb1.tile([128, 1024], F32R, name="accA")
                        accB = asb1.tile([128, 1024], F32R, name="accB")
                        ps_tiles = {}

                        def emit_sc(kt):
                            k0 = kt * 128
                            ps = pSC.tile([128, 1024], F32, name="sc")
                            for j in range(2):
                                nc.tensor.matmul(
                                    ps[:, j * 512:(j + 1) * 512],
                                    k_t[:, k0:k0 + 128],
                                    q_t[h][:, c0 + j * 512:c0 + (j + 1) * 512],
                                    start=True, stop=True)
                            ps_tiles[kt] = ps

                        pipe_sc = opt.get("pipe_sc", True)
                        if pipe_sc:
                            emit_sc(0)
                        for kt in range(KT):
                            k0 = kt * 128
                            # keep the PE stream fed: next scores are emitted
                            # before this kt's AV (which waits on the exp)
                            if pipe_sc:
                                if kt + 1 < KT:
                                    emit_sc(kt + 1)
                            else:
                                emit_sc(kt)
                            ps = ps_tiles.pop(kt)
                            et = asb3.tile([128, 1024], BF16, name="expt")
                            nc.scalar.activation(et[:, :], ps[:, :], AF.Exp,
                                                 bias=0.0,
                                                 scale=nk_col[:, kt:kt + 1])
                            for j in range(2):
                                nc.tensor.matmul(po[j][:, :], vnat[:, k0:k0 + 128],
                                                 et[:, j * 512:(j + 1) * 512],
                                                 start=(kt == 0), stop=(kt == KT - 1))
                            if kt == 0:
                                nc.gpsimd.tensor_copy(accA[:, :], et[:, :])
                            elif kt == 1:
                                nc.vector.tensor_copy(accB[:, :], et[:, :])
                            elif kt % 2 == 0:
                                nc.gpsimd.tensor_add(accA[:, :], accA[:, :],
                                                     et[:, :])
                            else:
                                nc.vector.tensor_add(accB[:, :], accB[:, :],
                                                     et[:, :])
                        o_tmp = asb2.tile([128, 1024], F32, name="o_tmp")
                        for j in range(2):
                            s0 = c0 + j * 512
                            # drain AV psum right away (f32 staging so the
                            # gate/recip scale rounds to bf16 only once);
                            # Pool cannot read PSUM, so DVE takes this
                            nc.vector.tensor_copy(o_tmp[:, j * 512:(j + 1) * 512],
                                                  po[j][:, :])
                            srow = pRow.tile([1, 512], F32, name="row")
                            nc.tensor.matmul(srow[:, :], t_ones[:, :],
                                             accA[:, j * 512:(j + 1) * 512],
                                             start=True, stop=False)
                            nc.tensor.matmul(srow[:, :], t_ones[:, :],
                                             accB[:, j * 512:(j + 1) * 512],
                                             start=False, stop=True)
                            rcp = asb2.tile([1, 512], F32, name="rcp")
                            nc.vector.reciprocal(rcp[:, :], srow[:, :])
                            grow = asb2.tile([1, 512], F32, name="grow")
                            nc.sync.dma_start(grow[:, :],
                                              gate_scr[h:h + 1, s0:s0 + 512])
                            rcb = asb2.tile([1, 512], F32, name="rcb")
                            nc.vector.tensor_mul(rcb[:, :], rcp[:, :], grow[:, :])
                            nc.sync.dma_start(scale_scr[h:h + 1, s0:s0 + 512],
                                              rcb[:, :])
                            sb = asb2.tile([128, 512], F32, name="scaleb")
                            nc.sync.dma_start(
                                sb[:, :],
                                scale_scr[h:h + 1, s0:s0 + 512].to_broadcast(
                                    (128, 512)))
                            nc.vector.tensor_mul(o_t[h][:, s0:s0 + 512],
                                                 o_tmp[:, j * 512:(j + 1) * 512],
                                                 sb[:, :])

                if opt.get("pipe_rope", True):
                    emit_rope(0)
                    for h in range(HPC):
                        if h + 1 < HPC:
                            emit_rope(h + 1)
                        emit_attn(h)
                else:
                    for h in range(HPC):
                        emit_rope(h)
                        emit_attn(h)

            # ============ Phase 4: projection ===========================
            with tc.tile_pool(name=f"pj_ps{_rep}", bufs=4, space="PSUM") as pPJ, \
                 tc.tile_pool(name=f"pj_sb{_rep}", bufs=2) as pjs:
                NTP = HID // 1024  # 5 pairs of 512-wide tiles
                wt_sets = {}

                def emit_wt(ntp):
                    n0 = ntp * 1024
                    wt = [pjs.tile([128, 1024], BF16, name=f"wp{h}")
                          for h in range(HPC)]
                    for h in range(HPC):
                        nc.sync.dma_start(wt[h][:, :], wp[h, :, n0:n0 + 1024])
                    wt_sets[ntp] = wt

                pipe_wt = opt.get("pipe_wt", True)
                if pipe_wt:
                    emit_wt(0)
                for ntp in range(NTP):
                    n0 = ntp * 1024
                    if pipe_wt:
                        if ntp + 1 < NTP:
                            emit_wt(ntp + 1)
                    else:
                        emit_wt(ntp)
                    wt = wt_sets.pop(ntp)
                    for st in range(ST):
                        s0 = st * 128
                        ob = pjs.tile([128, 1024], F32, name="outsb")
                        for j in range(2):
                            pp = pPJ.tile([128, 512], F32, name=f"pj{j}")
                            for h in range(HPC):
                                nc.tensor.matmul(pp[:, :], o_t[h][:, s0:s0 + 128],
                                                 wt[h][:, j * 512:(j + 1) * 512],
                                                 start=(h == 0), stop=(h == HPC - 1))
                            # alternate psum drains between DVE and ACT
                            # (Pool cannot read PSUM)
                            if j == 0:
                                nc.vector.tensor_copy(ob[:, j * 512:(j + 1) * 512],
                                                      pp[:, :])
                            else:
                                nc.scalar.copy(ob[:, j * 512:(j + 1) * 512],
                                               pp[:, :])
                        # ACT's queue: keeps the big stores off the sync queue
                        # so the next repeat's x/w loads aren't stuck behind
                        # them
                        nc.scalar.dma_start(out[s0:s0 + 128, n0:n0 + 1024],
                                            ob[:, :])
    nc.finalize()
    return nc


# ---------------- host-side prep & execution ----------------

_CACHE = {}


def _get_exec(repeat=1):
    key = (repeat, tuple(sorted(BUILD_OPTS.items())))
    if key in _CACHE:
        return _CACHE[key]

    import jax
    from concourse import bass2jax, mybir as mb
    from jax.experimental.shard_map import shard_map
    from jax.sharding import Mesh, PartitionSpec

    bass2jax.install_neuronx_cc_hook()
    nc = build_program(repeat)

    part_name = nc.partition_id_tensor.name if nc.partition_id_tensor else None
    in_names, out_names, out_avals = [], [], []
    for alloc in nc.m.functions[0].allocations:
        if not isinstance(alloc, mb.MemoryLocationSet):
            continue
        name = alloc.memorylocations[0].name
        if alloc.kind == "ExternalInput":
            if name != part_name:
                in_names.append(name)
        elif alloc.kind == "ExternalOutput":
            out_names.append(name)
            out_avals.append(jax.core.ShapedArray(tuple(alloc.tensor_shape),
                                                  mb.dt.np(alloc.dtype)))
    n_params = len(in_names)
    all_names = in_names + out_names
    if part_name is not None:
        all_names = all_names + [part_name]

    def _body(*args):
        operands = list(args)
        if part_name is not None:
            operands.append(bass2jax.partition_id_tensor())
        outs = bass2jax._bass_exec_p.bind(
            *operands,
            out_avals=tuple(out_avals),
            in_names=tuple(all_names),
            out_names=tuple(out_names),
            lowering_input_output_aliases=(),
            sim_require_finite=True,
            sim_require_nnan=True,
            nc=nc,
        )
        return tuple(outs)

    devices = jax.devices()[:NC]
    mesh = Mesh(np.asarray(devices), ("core",))
    spec = (PartitionSpec("core"),) * (n_params + len(out_names))
    fn = jax.jit(shard_map(_body, mesh=mesh, in_specs=spec,
                           out_specs=(PartitionSpec("core"),) * len(out_names),
                           check_rep=False), keep_unused=True)
    _CACHE[key] = dict(fn=fn, nc=nc, in_names=in_names, out_names=out_names,
                       out_avals=out_avals, mesh=mesh)
    return _CACHE[key]


def prep_inputs(x, rope_cos, rope_sin, w_pre_norm, w_qkv, w_q_norm, w_k_norm,
                w_proj):
    """Build the per-core input dict list (host-side sharding/layout only)."""
    x = np.asarray(x, np.float32)
    w_qkv = np.asarray(w_qkv, np.float32)
    w_proj = np.asarray(w_proj, np.float32)
    w_pre = np.asarray(w_pre_norm, np.float32)
    w_qn = np.asarray(w_q_norm, np.float32)
    w_kn = np.asarray(w_k_norm, np.float32)
    cos = np.asarray(rope_cos, np.float32)[0]   # [S, D]
    sin = np.asarray(rope_sin, np.float32)[0]

    bf16 = ml_dtypes.bfloat16
    xT = np.ascontiguousarray(x[0].T).reshape(HT, 128, S).astype(bf16)

    cosT = np.ascontiguousarray(cos.T)          # [D, S]
    sinT = np.ascontiguousarray(sin.T)
    sign = np.where(np.arange(D) < D // 2, -1.0, 1.0).astype(np.float32)

    def rope_tables(w):
        w_swap = np.concatenate([w[D // 2:], w[:D // 2]])
        c = cosT * w[:, None]
        s = sinT * (sign * w_swap)[:, None]
        return (np.ascontiguousarray(c).astype(bf16),
                np.ascontiguousarray(s).astype(bf16))

    cq, sq_ = rope_tables(w_qn)
    ck, sk = rope_tables(w_kn)

    wqkv_eff = w_pre[:, None] * w_qkv           # fold pre-norm weight (exact)
    q_dim, k_dim = NQ * D, NKV * D
    ones = np.ones((128, 1), np.float32)
    ident = np.eye(128, dtype=np.float32).astype(bf16)

    in_maps = []
    for c in range(NC):
        wslice = np.concatenate([
            wqkv_eff[:, (HPC * c) * D:(HPC * c + HPC) * D],
            wqkv_eff[:, q_dim + c * D:q_dim + (c + 1) * D],
            wqkv_eff[:, q_dim + k_dim + c * D:q_dim + k_dim + (c + 1) * D],
            wqkv_eff[:, q_dim + 2 * k_dim + HPC * c:q_dim + 2 * k_dim + HPC * (c + 1)],
        ], axis=1)                               # [HID, 901]
        wslice = np.ascontiguousarray(wslice).reshape(
            HT, 128, QKV_COLS).astype(bf16)
        wpslice = np.ascontiguousarray(
            w_proj[(HPC * c) * D:(HPC * c + HPC) * D, :]).reshape(
                HPC, 128, HID).astype(bf16)
        in_maps.append({
            "xT": xT, "wq": wslice, "wp": wpslice,
            "cosq": cq, "sinq": sq_, "cosk": ck, "sink": sk,
            "ones_col": ones, "ident": ident,
        })
    return in_maps


def run_in_maps(in_maps):
    """Execute the SPMD program; returns list of per-core {out: [S, HID]}."""
    cache = _get_exec()
    fn, in_names, out_names, out_avals = (cache["fn"], cache["in_names"],
                                          cache["out_names"], cache["out_avals"])
    concat_in = [np.concatenate([m[nm] for m in in_maps], axis=0)
                 for nm in in_names]
    zeros = [np.zeros((NC * a.shape[0], *a.shape[1:]), a.dtype) for a in out_avals]
    outs = fn(*concat_in, *zeros)
    res = []
    for c in range(NC):
        d = {}
        for i, nm in enumerate(out_names):
            shp = out_avals[i].shape
            d[nm] = np.asarray(outs[i]).reshape(NC, *shp)[c]
        res.append(d)
    return res


def kernel(**inputs):
    in_maps = prep_inputs(**inputs)
    res = run_in_maps(in_maps)
    total = res[0]["out"].astype(np.float32)
    for c in range(1, NC):
        total = total + res[c]["out"]
    return total.reshape(1, S, HID)


# revision 52
# speedup vs baseline: 1.7265x; 1.7265x over previous
"""Trainium2 Bass kernel for a GQA attention block (RMSNorm -> QKV+gate ->
Q/K-norm -> RoPE -> attention -> gated out -> proj), tensor-parallel over
heads across 8 NeuronCores.

Sharding: core c owns q heads [5c, 5c+5) and kv group c (NQ=40, NKV=8).
Each core computes a partial projection output; partials are summed on host
(row-parallel proj unshard).

v1: bf16 matmul operands everywhere (f32 PSUM accumulate, f32 softmax /
norm statistics), qkv weights and rope tables resident in SBUF.
"""
import sys

sys.path.insert(0, "/opt/trn_rl_repo")

import ml_dtypes
import numpy as np

import bass_rust as _bass_rust

import concourse.bacc as bacc
import concourse.tile as tile
from concourse import mybir
from concourse.hw_specs import get_activation_tables


class _Bacc(bacc.Bacc):
    """Bacc with activation-table choice restricted to the exp+ln set.

    The stock insert_act_table_loads pass picks the FIRST act_func_set
    containing each function, so alternating Ln/Exp activations thrash
    between two tables.  Emptying all sets except `natural_log_exp_and_others`
    (square/copy/exp/ln) and `sigmoid_and_others` forces one resident table
    for the whole kernel (plus a single swap around the sigmoid).
    """

    _KEEP_SETS = {"natural_log_exp_and_others", "sigmoid_and_others"}

    def insert_act_table_loads(self):
        has_activation = any(
            isinstance(i, mybir.InstActivation)
            for b in self.main_func.blocks
            for i in b.instructions
        )
        if not has_activation:
            return
        tables = [
            (name, (fns if name in self._KEEP_SETS else set()))
            for name, fns in get_activation_tables(self.m.arch).items()
        ]
        _bass_rust.insert_act_table_loads(self, tables)

NQ, NKV, D, HID = 40, 8, 128, 5120
S = 2048
NC = 8
HPC = NQ // NC          # q heads per core = 5
EPS = 1e-6
HT = HID // 128         # 40 hid tiles
ST = S // 128           # 16 seq tiles of 128
NCH = S // 512          # 4 chunks of 512
KT = S // 128           # 16 k-tiles
QKV_COLS = HPC * D + 2 * D + HPC   # 901
F32 = mybir.dt.float32
F32R = mybir.dt.float32r
BF16 = mybir.dt.bfloat16
AF = mybir.ActivationFunctionType
# build-time tunables (A/B-tested defaults are read inside build_program)
BUILD_OPTS = {}


def build_program(repeat=1):
    opt = BUILD_OPTS
    nc = _Bacc(None, target_bir_lowering=False)

    # register activation-bias constants (mirrors Bass.__init__ registration)
    for val in (EPS, float(D) * EPS):
        t = nc.alloc_sbuf_tensor(f"const-float32-{val}", [128, 1], F32)
        nc.gpsimd.memset(t.ap(), val)
        nc.const_aps.aps[(F32, val)] = t.ap()
    nc.all_engine_barrier()

    # ---- I/O ----
    xT = nc.dram_tensor("xT", [HT, 128, S], BF16, kind="ExternalInput")
    wq = nc.dram_tensor("wq", [HT, 128, QKV_COLS], BF16, kind="ExternalInput")
    wp = nc.dram_tensor("wp", [HPC, 128, HID], BF16, kind="ExternalInput")
    cosq = nc.dram_tensor("cosq", [128, S], BF16, kind="ExternalInput")
    sinq = nc.dram_tensor("sinq", [128, S], BF16, kind="ExternalInput")
    cosk = nc.dram_tensor("cosk", [128, S], BF16, kind="ExternalInput")
    sink = nc.dram_tensor("sink", [128, S], BF16, kind="ExternalInput")
    ones_col = nc.dram_tensor("ones_col", [128, 1], F32R, kind="ExternalInput")
    ident = nc.dram_tensor("ident", [128, 128], BF16, kind="ExternalInput")
    out = nc.dram_tensor("out", [S, HID], F32, kind="ExternalOutput")

    with tile.TileContext(nc, pool_alloc_mode=opt.get("palloc", "stack")) as tc:
      for _rep in range(repeat):
        with tc.tile_pool(name=f"persist{_rep}", bufs=1) as pers, \
             tc.tile_pool(name=f"cols{_rep}", bufs=1) as cols, \
             tc.tile_pool(name=f"scr{_rep}", bufs=1, space="DRAM") as dscr:
            # DRAM row bounces (for partition-broadcast / row->col reshape)
            lnm_scr = dscr.tile([1, S], F32, name="lnm_scr")
            lnk_scr = dscr.tile([1, S], F32, name="lnk_scr")
            rrow_scr = dscr.tile([1, S], F32, name="rrow_scr")
            gate_scr = dscr.tile([HPC, S], F32, name="gate_scr")
            nq_scr = dscr.tile([HPC, S], F32, name="nq_scr")
            scale_scr = dscr.tile([HPC, S], F32, name="scale_scr")
            # persistent small tiles
            t_ones = cols.tile([128, 1], F32R, name="ones")
            nc.sync.dma_start(t_ones[:, :], ones_col[:, :])
            t_id = cols.tile([128, 128], BF16, name="ident")
            nc.sync.dma_start(t_id[:, :], ident[:, :])
            t_ones_b = cols.tile([128, 1], BF16, name="ones_b")
            nc.gpsimd.tensor_copy(t_ones_b[:, :], t_ones[:, :])


            # resident qkv weights (bf16) and rope tables.  The weight loads
            # are interleaved with the first chunk's x loads inside phase 1
            # (same FIFO queue) so the PE isn't starved at startup; the rope
            # tables ride a different engine's queue.
            wq_sb = pers.tile([128, HT, QKV_COLS], BF16, name="wq_sb")
            t_cosq = pers.tile([128, S], BF16, name="cosq")
            t_sinq = pers.tile([128, S], BF16, name="sinq")
            t_cosk = pers.tile([128, S], BF16, name="cosk")
            t_sink = pers.tile([128, S], BF16, name="sink")
            nc.scalar.dma_start(t_cosq[:, :], cosq[:, :])
            nc.scalar.dma_start(t_sinq[:, :], sinq[:, :])
            nc.scalar.dma_start(t_cosk[:, :], cosk[:, :])
            nc.scalar.dma_start(t_sink[:, :], sink[:, :])

            q_t = [pers.tile([128, S], BF16, name=f"q{h}") for h in range(HPC)]
            k_t = pers.tile([128, S], BF16, name="kT")
            vnat = pers.tile([128, S], BF16, name="vnat")
            o_t = [pers.tile([128, S], BF16, name=f"o{h}") for h in range(HPC)]

            with tc.tile_pool(name=f"vg{_rep}", bufs=1) as vg:
                v_t = vg.tile([128, S], BF16, name="vT")
                gates = vg.tile([5, S], F32, name="gates")

                # ============ Phase 1: QKV matmul (+ x^2 accumulation) ======
                with tc.tile_pool(name=f"qkv_ps{_rep}", bufs=1, space="PSUM") as psA, \
                     tc.tile_pool(name=f"qkv_psB{_rep}", bufs=1, space="PSUM") as psB, \
                     tc.tile_pool(name=f"qkv_sb{_rep}", bufs=opt.get("qkv_bufs", 3)) as sb2, \
                     tc.tile_pool(name=f"qkv_sb3{_rep}", bufs=3) as sb3:
                    for ch in range(NCH):
                        c0 = ch * 512
                        pm = [psA.tile([128, 512], F32, name=f"mt{m}")
                              for m in range(7)]
                        pg = psB.tile([5, 512], F32, name="small")
                        acc = sb2.tile([128, 1024], F32, name="accsq")
                        accr = sb2.tile([128, 512], F32R, name="accr")
                        NP = HT // 2
                        for hp in range(NP):
                            ht = 2 * hp
                            if ch == 0:
                                nc.sync.dma_start(
                                    wq_sb[:, ht:ht + 2, :],
                                    wq[ht:ht + 2, :, :].rearrange(
                                        "t p c -> p t c"))
                            xt = sb2.tile([128, 2, 512], BF16, name="xt")
                            nc.sync.dma_start(
                                xt[:, :, :],
                                xT[ht:ht + 2, :, c0:c0 + 512].rearrange(
                                    "t p c -> p t c"))
                            for i in range(2):
                                for m in range(7):
                                    nc.tensor.matmul(
                                        pm[m][:, :],
                                        wq_sb[:, ht + i, m * 128:(m + 1) * 128],
                                        xt[:, i, :], start=(hp == 0 and i == 0),
                                        stop=(hp == NP - 1 and i == 1))
                                nc.tensor.matmul(pg[:, :],
                                                 wq_sb[:, ht + i, 896:901],
                                                 xt[:, i, :],
                                                 start=(hp == 0 and i == 0),
                                                 stop=(hp == NP - 1 and i == 1))
                            sq = sb3.tile([128, 1024], F32, name="sq")
                            nc.scalar.activation(
                                sq[:, :],
                                xt[:, :, :].rearrange("p t c -> p (t c)"),
                                AF.Square)
                            if hp == 0:
                                nc.gpsimd.tensor_copy(acc[:, :], sq[:, :])
                            else:
                                nc.gpsimd.tensor_add(acc[:, :], acc[:, :], sq[:, :])
                        nc.vector.tensor_add(accr[:, :], acc[:, 0:512],
                                             acc[:, 512:1024])
                        use_split = opt.get("drain_split", True)
                        for h in range(HPC):
                            if use_split and h in (1, 3):
                                nc.scalar.copy(q_t[h][:, c0:c0 + 512], pm[h][:, :])
                            else:
                                nc.vector.tensor_copy(q_t[h][:, c0:c0 + 512],
                                                      pm[h][:, :])
                        if use_split:
                            nc.scalar.copy(k_t[:, c0:c0 + 512], pm[5][:, :])
                        else:
                            nc.vector.tensor_copy(k_t[:, c0:c0 + 512], pm[5][:, :])
                        nc.vector.tensor_copy(v_t[:, c0:c0 + 512], pm[6][:, :])
                        nc.vector.tensor_copy(gates[:, c0:c0 + 512], pg[:, :])
                        pr = psB.tile([1, 512], F32, name="small")
                        nc.tensor.matmul(pr[:, :], t_ones[:, :], accr[:, :],
                                         start=True, stop=True)
                        lnm_row = sb2.tile([1, 512], F32, name="lnm_row")
                        nc.scalar.activation(lnm_row[:, :], pr[:, :], AF.Ln,
                                             bias=EPS, scale=1.0 / HID)
                        nc.sync.dma_start(lnm_scr[0:1, c0:c0 + 512], lnm_row[:, :])
                        r_row = sb2.tile([1, 512], F32, name="r_row")
                        nc.scalar.activation(r_row[:, :], lnm_row[:, :], AF.Exp,
                                             bias=0.0, scale=-0.5)
                        nc.sync.dma_start(rrow_scr[0:1, c0:c0 + 512], r_row[:, :])

                # ============ Phase 2a: v transpose (x r) + gates ===========
                with tc.tile_pool(name=f"tr_ps{_rep}", bufs=2, space="PSUM") as pTr, \
                     tc.tile_pool(name=f"ga_sb{_rep}", bufs=2) as gsb:
                    lnm_col = cols.tile([128, KT], F32, name="lnm_col")
                    nc.sync.dma_start(lnm_col[:, :],
                                      lnm_scr[0, :].rearrange("(t p) -> p t", p=128))
                    r_col = cols.tile([128, KT], F32, name="r_col")
                    nc.scalar.activation(r_col[:, :], lnm_col[:, :], AF.Exp,
                                         bias=0.0, scale=-0.5)
                    for kt in range(KT):
                        ptr = pTr.tile([128, 128], BF16, name="tr")
                        nc.tensor.transpose(ptr[:, :], v_t[:, kt * 128:(kt + 1) * 128],
                                            t_id[:, :])
                        nc.vector.tensor_scalar_mul(vnat[:, kt * 128:(kt + 1) * 128],
                                                    ptr[:, :], r_col[:, kt:kt + 1])
                    for ch in range(NCH):
                        c0 = ch * 512
                        rb = gsb.tile([5, 512], F32, name="rhatb")
                        nc.sync.dma_start(
                            rb[:, :],
                            rrow_scr[0:1, c0:c0 + 512].to_broadcast((5, 512)))
                        nc.vector.tensor_mul(gates[:, c0:c0 + 512],
                                             gates[:, c0:c0 + 512], rb[:, :])
                    nc.scalar.activation(gates[:, :], gates[:, :], AF.Sigmoid)
                    nc.sync.dma_start(gate_scr[:, :], gates[:, :])
                    # ---- k: norm sums + rope here so they overlap the gate
                    # and v work instead of serializing at attention start
                    for ch in range(NCH):
                        c0 = ch * 512
                        ksq = gsb.tile([128, 512], BF16, name="ksqc")
                        nc.scalar.activation(ksq[:, :], k_t[:, c0:c0 + 512],
                                             AF.Square)
                        pn = pTr.tile([1, 512], F32, name="prow")
                        nc.tensor.matmul(pn[:, :], t_ones_b[:, :], ksq[:, :],
                                         start=True, stop=True)
                        lnk_row = gsb.tile([1, 512], F32, name="lnkrow")
                        nc.scalar.activation(lnk_row[:, :], pn[:, :], AF.Ln,
                                             bias=D * EPS, scale=1.0)
                        nc.sync.dma_start(lnk_scr[0:1, c0:c0 + 512],
                                          lnk_row[:, :])
                    for c2 in range(2):
                        c0 = c2 * 1024
                        rot = gsb.tile([128, 1024], BF16, name="rotk")
                        nc.sync.dma_start(rot[0:64, :], k_t[64:128, c0:c0 + 1024])
                        nc.sync.dma_start(rot[64:128, :], k_t[0:64, c0:c0 + 1024])
                        t1 = gsb.tile([128, 1024], F32, name="t1k")
                        rr = gsb.tile([128, 1024], F32, name="rrk")
                        nc.vector.tensor_mul(t1[:, :], k_t[:, c0:c0 + 1024],
                                             t_cosk[:, c0:c0 + 1024])
                        nc.vector.tensor_mul(rr[:, :], rot[:, :],
                                             t_sink[:, c0:c0 + 1024])
                        nc.vector.tensor_add(k_t[:, c0:c0 + 1024], t1[:, :],
                                             rr[:, :])
            # vg pool (v_t, gates) closes here

            # ============ Phase 2b/3: norms, rope, attention ================
            with tc.tile_pool(name=f"at_row{_rep}", bufs=2, space="PSUM") as pRow, \
                 tc.tile_pool(name=f"at_sc{_rep}", bufs=2, space="PSUM") as pSC, \
                 tc.tile_pool(name=f"at_av{_rep}", bufs=1, space="PSUM") as pAV, \
                 tc.tile_pool(name=f"at_sb1{_rep}", bufs=opt.get("acc_bufs", 1)) as asb1, \
                 tc.tile_pool(name=f"at_sb2{_rep}", bufs=2) as asb2, \
                 tc.tile_pool(name=f"at_rp{_rep}", bufs=1) as rsb, \
                 tc.tile_pool(name=f"at_sb3{_rep}", bufs=opt.get("expt_bufs", 5)) as asb3:

                lnk_col = cols.tile([128, KT], F32, name="lnk_col")
                nc.sync.dma_start(lnk_col[:, :],
                                  lnk_scr[0, :].rearrange("(t p) -> p t", p=128))
                nk_col = cols.tile([128, KT], F32, name="nk_col")
                nc.scalar.activation(nk_col[:, :], lnk_col[:, :], AF.Exp,
                                     bias=0.0, scale=-0.5)

                # ---- per head: norm + rope, then attention, software-
                # pipelined: rope(h+1) is emitted before attention(h) so the
                # in-order DVE stream ropes the next head while the PE works
                # on the current head's attention.
                def emit_rope(h):
                    for c2 in range(2):
                        c0 = c2 * 1024
                        for j in range(2):
                            s0 = c0 + j * 512
                            qsq = asb2.tile([128, 512], BF16, name="sqc")
                            # square on DVE (mul) — ACT is the bottleneck in
                            # the attention phase
                            nc.vector.tensor_mul(qsq[:, :], q_t[h][:, s0:s0 + 512],
                                                 q_t[h][:, s0:s0 + 512])
                            pn = pRow.tile([1, 512], F32, name="row")
                            nc.tensor.matmul(pn[:, :], t_ones_b[:, :], qsq[:, :],
                                             start=True, stop=True)
                            lnq = asb2.tile([1, 512], F32, name="lnq")
                            nc.scalar.activation(lnq[:, :], pn[:, :], AF.Ln,
                                                 bias=EPS, scale=1.0 / D)
                            nqr = asb2.tile([1, 512], F32, name="nqrow")
                            nc.scalar.activation(nqr[:, :], lnq[:, :], AF.Exp,
                                                 bias=0.0, scale=-0.5)
                            nc.sync.dma_start(nq_scr[h:h + 1, s0:s0 + 512], nqr[:, :])
                        nb = asb2.tile([128, 1024], F32, name="nb")
                        nc.sync.dma_start(
                            nb[:, :],
                            nq_scr[h:h + 1, c0:c0 + 1024].to_broadcast((128, 1024)))
                        rot = asb2.tile([128, 1024], BF16, name="rot")
                        nc.sync.dma_start(rot[0:64, :], q_t[h][64:128, c0:c0 + 1024])
                        nc.sync.dma_start(rot[64:128, :], q_t[h][0:64, c0:c0 + 1024])
                        t1 = rsb.tile([128, 1024], F32, name="t1")
                        rr = rsb.tile([128, 1024], F32, name="rr")
                        nc.vector.tensor_mul(t1[:, :], q_t[h][:, c0:c0 + 1024],
                                             t_cosq[:, c0:c0 + 1024])
                        nc.vector.tensor_mul(rr[:, :], rot[:, :],
                                             t_sinq[:, c0:c0 + 1024])
                        nc.vector.tensor_add(t1[:, :], t1[:, :], rr[:, :])
                        nc.vector.tensor_mul(q_t[h][:, c0:c0 + 1024], t1[:, :],
                                             nb[:, :])

                def emit_attn(h):
                    for qp in range(2):
                        c0 = qp * 1024
                        po = [pAV.tile([128, 512], F32, name=f"av{j}")
                              for j in range(2)]
                        accA = asb1.tile([128, 1024], F32R, name="accA")
                        accB = asb1.tile([128, 1024], F32R, name="accB")
                        ps_tiles = {}

                        def emit_sc(kt):
                            k0 = kt * 128
                            ps = pSC.tile([128, 1024], F32, name="sc")
                            for j in range(2):
                                nc.tensor.matmul(
                                    ps[:, j * 512:(j + 1) * 512],
                                    k_t[:, k0:k0 + 128],
                                    q_t[h][:, c0 + j * 512:c0 + (j + 1) * 512],
                                    start=True, stop=True)
                            ps_tiles[kt] = ps

                        pipe_sc = opt.get("pipe_sc", True)
                        if pipe_sc:
                            emit_sc(0)
                        for kt in range(KT):
                            k0 = kt * 128
                            # keep the PE stream fed: next scores are emitted
                            # before this kt's AV (which waits on the exp)
                            if pipe_sc:
                                if kt + 1 < KT:
                                    emit_sc(kt + 1)
                            else:
                                emit_sc(kt)
                            ps = ps_tiles.pop(kt)
                            et = asb3.tile([128, 1024], BF16, name="expt")
                            nc.scalar.activation(et[:, :], ps[:, :], AF.Exp,
                                                 bias=0.0,
                                                 scale=nk_col[:, kt:kt + 1])
                            for j in range(2):
                                nc.tensor.matmul(po[j][:, :], vnat[:, k0:k0 + 128],
                                                 et[:, j * 512:(j + 1) * 512],
                                                 start=(kt == 0), stop=(kt == KT - 1))
                            # Pool is otherwise idle in this phase while
                            # DVE also carries rope/drains/recip/scales, so
                            # Pool takes 12 of the 16 accumulate ops
                            if kt == 0:
                                nc.gpsimd.tensor_copy(accA[:, :], et[:, :])
                            elif kt == 3:
                                nc.vector.tensor_copy(accB[:, :], et[:, :])
                            elif kt % 4 != 3:
                                nc.gpsimd.tensor_add(accA[:, :], accA[:, :],
                                                     et[:, :])
                            else:
                                nc.vector.tensor_add(accB[:, :], accB[:, :],
                                                     et[:, :])
                        o_tmp = asb2.tile([128, 1024], F32, name="o_tmp")
                        for j in range(2):
                            s0 = c0 + j * 512
                            # drain AV psum right away (f32 staging so the
                            # gate/recip scale rounds to bf16 only once);
                            # Pool cannot read PSUM, so DVE takes this
                            nc.vector.tensor_copy(o_tmp[:, j * 512:(j + 1) * 512],
                                                  po[j][:, :])
                            srow = pRow.tile([1, 512], F32, name="row")
                            nc.tensor.matmul(srow[:, :], t_ones[:, :],
                                             accA[:, j * 512:(j + 1) * 512],
                                             start=True, stop=False)
                            nc.tensor.matmul(srow[:, :], t_ones[:, :],
                                             accB[:, j * 512:(j + 1) * 512],
                                             start=False, stop=True)
                            rcp = asb2.tile([1, 512], F32, name="rcp")
                            nc.vector.reciprocal(rcp[:, :], srow[:, :])
                            grow = asb2.tile([1, 512], F32, name="grow")
                            nc.sync.dma_start(grow[:, :],
                                              gate_scr[h:h + 1, s0:s0 + 512])
                            rcb = asb2.tile([1, 512], F32, name="rcb")
                            nc.vector.tensor_mul(rcb[:, :], rcp[:, :], grow[:, :])
                            nc.sync.dma_start(scale_scr[h:h + 1, s0:s0 + 512],
                                              rcb[:, :])
                            sb = asb2.tile([128, 512], F32, name="scaleb")
                            nc.sync.dma_start(
                                sb[:, :],
                                scale_scr[h:h + 1, s0:s0 + 512].to_broadcast(
                                    (128, 512)))
                            nc.vector.tensor_mul(o_t[h][:, s0:s0 + 512],
                                                 o_tmp[:, j * 512:(j + 1) * 512],
                                                 sb[:, :])

                if opt.get("pipe_rope", True):
                    emit_rope(0)
                    for h in range(HPC):
                        if h + 1 < HPC:
                            emit_rope(h + 1)
                        emit_attn(h)
                else:
                    for h in range(HPC):
                        emit_rope(h)
                        emit_attn(h)

            # ============ Phase 4: projection ===========================
            with tc.tile_pool(name=f"pj_ps{_rep}", bufs=4, space="PSUM") as pPJ, \
                 tc.tile_pool(name=f"pj_sb{_rep}", bufs=2) as pjs:
                NTP = HID // 1024  # 5 pairs of 512-wide tiles
                wt_sets = {}

                def emit_wt(ntp):
                    n0 = ntp * 1024
                    wt = [pjs.tile([128, 1024], BF16, name=f"wp{h}")
                          for h in range(HPC)]
                    for h in range(HPC):
                        nc.sync.dma_start(wt[h][:, :], wp[h, :, n0:n0 + 1024])
                    wt_sets[ntp] = wt

                pipe_wt = opt.get("pipe_wt", True)
                if pipe_wt:
                    emit_wt(0)
                for ntp in range(NTP):
                    n0 = ntp * 1024
                    if pipe_wt:
                        if ntp + 1 < NTP:
                            emit_wt(ntp + 1)
                    else:
                        emit_wt(ntp)
                    wt = wt_sets.pop(ntp)
                    for st in range(ST):
                        s0 = st * 128
                        ob = pjs.tile([128, 1024], F32, name="outsb")
                        for j in range(2):
                            pp = pPJ.tile([128, 512], F32, name=f"pj{j}")
                            for h in range(HPC):
                                nc.tensor.matmul(pp[:, :], o_t[h][:, s0:s0 + 128],
                                                 wt[h][:, j * 512:(j + 1) * 512],
                                                 start=(h == 0), stop=(h == HPC - 1))
                            # alternate psum drains between DVE and ACT
                            # (Pool cannot read PSUM)
                            if j == 0:
                                nc.vector.tensor_copy(ob[:, j * 512:(j + 1) * 512],
                                                      pp[:, :])
                            else:
                                nc.scalar.copy(ob[:, j * 512:(j + 1) * 512],
                                               pp[:, :])
                        # ACT's queue: keeps the big stores off the sync queue
                        # so the next repeat's x/w loads aren't stuck behind
                        # them
                        nc.scalar.dma_start(out[s0:s0 + 128, n0:n0 + 1024],
                                            ob[:, :])
    nc.finalize()
    return nc


# ---------------- host-side prep & execution ----------------

_CACHE = {}


def _get_exec(repeat=1):
    key = (repeat, tuple(sorted(BUILD_OPTS.items())))
    if key in _CACHE:
        return _CACHE[key]

    import jax
    from concourse import bass2jax, mybir as mb
    from jax.experimental.shard_map import shard_map
    from jax.sharding import Mesh, PartitionSpec

    bass2jax.install_neuronx_cc_hook()
    nc = build_program(repeat)

    part_name = nc.partition_id_tensor.name if nc.partition_id_tensor else None
    in_names, out_names, out_avals = [], [], []
    for alloc in nc.m.functions[0].allocations:
        if not isinstance(alloc, mb.MemoryLocationSet):
            continue
        name = alloc.memorylocations[0].name
        if alloc.kind == "ExternalInput":
            if name != part_name:
                in_names.append(name)
        elif alloc.kind == "ExternalOutput":
            out_names.append(name)
            out_avals.append(jax.core.ShapedArray(tuple(alloc.tensor_shape),
                                                  mb.dt.np(alloc.dtype)))
    n_params = len(in_names)
    all_names = in_names + out_names
    if part_name is not None:
        all_names = all_names + [part_name]

    def _body(*args):
        operands = list(args)
        if part_name is not None:
            operands.append(bass2jax.partition_id_tensor())
        outs = bass2jax._bass_exec_p.bind(
            *operands,
            out_avals=tuple(out_avals),
            in_names=tuple(all_names),
            out_names=tuple(out_names),
            lowering_input_output_aliases=(),
            sim_require_finite=True,
            sim_require_nnan=True,
            nc=nc,
        )
        return tuple(outs)

    devices = jax.devices()[:NC]
    mesh = Mesh(np.asarray(devices), ("core",))
    spec = (PartitionSpec("core"),) * (n_params + len(out_names))
    fn = jax.jit(shard_map(_body, mesh=mesh, in_specs=spec,
                           out_specs=(PartitionSpec("core"),) * len(out_names),
                           check_rep=False), keep_unused=True)
    _CACHE[key] = dict(fn=fn, nc=nc, in_names=in_names, out_names=out_names,
                       out_avals=out_avals, mesh=mesh)
    return _CACHE[key]


def prep_inputs(x, rope_cos, rope_sin, w_pre_norm, w_qkv, w_q_norm, w_k_norm,
                w_proj):
    """Build the per-core input dict list (host-side sharding/layout only)."""
    x = np.asarray(x, np.float32)
    w_qkv = np.asarray(w_qkv, np.float32)
    w_proj = np.asarray(w_proj, np.float32)
    w_pre = np.asarray(w_pre_norm, np.float32)
    w_qn = np.asarray(w_q_norm, np.float32)
    w_kn = np.asarray(w_k_norm, np.float32)
    cos = np.asarray(rope_cos, np.float32)[0]   # [S, D]
    sin = np.asarray(rope_sin, np.float32)[0]

    bf16 = ml_dtypes.bfloat16
    xT = np.ascontiguousarray(x[0].T).reshape(HT, 128, S).astype(bf16)

    cosT = np.ascontiguousarray(cos.T)          # [D, S]
    sinT = np.ascontiguousarray(sin.T)
    sign = np.where(np.arange(D) < D // 2, -1.0, 1.0).astype(np.float32)

    def rope_tables(w):
        w_swap = np.concatenate([w[D // 2:], w[:D // 2]])
        c = cosT * w[:, None]
        s = sinT * (sign * w_swap)[:, None]
        return (np.ascontiguousarray(c).astype(bf16),
                np.ascontiguousarray(s).astype(bf16))

    cq, sq_ = rope_tables(w_qn)
    ck, sk = rope_tables(w_kn)

    wqkv_eff = w_pre[:, None] * w_qkv           # fold pre-norm weight (exact)
    q_dim, k_dim = NQ * D, NKV * D
    ones = np.ones((128, 1), np.float32)
    ident = np.eye(128, dtype=np.float32).astype(bf16)

    in_maps = []
    for c in range(NC):
        wslice = np.concatenate([
            wqkv_eff[:, (HPC * c) * D:(HPC * c + HPC) * D],
            wqkv_eff[:, q_dim + c * D:q_dim + (c + 1) * D],
            wqkv_eff[:, q_dim + k_dim + c * D:q_dim + k_dim + (c + 1) * D],
            wqkv_eff[:, q_dim + 2 * k_dim + HPC * c:q_dim + 2 * k_dim + HPC * (c + 1)],
        ], axis=1)                               # [HID, 901]
        wslice = np.ascontiguousarray(wslice).reshape(
            HT, 128, QKV_COLS).astype(bf16)
        wpslice = np.ascontiguousarray(
            w_proj[(HPC * c) * D:(HPC * c + HPC) * D, :]).reshape(
                HPC, 128, HID).astype(bf16)
        in_maps.append({
            "xT": xT, "wq": wslice, "wp": wpslice,
            "cosq": cq, "sinq": sq_, "cosk": ck, "sink": sk,
            "ones_col": ones, "ident": ident,
        })
    return in_maps


def run_in_maps(in_maps):
    """Execute the SPMD program; returns list of per-core {out: [S, HID]}."""
    cache = _get_exec()
    fn, in_names, out_names, out_avals = (cache["fn"], cache["in_names"],
                                          cache["out_names"], cache["out_avals"])
    concat_in = [np.concatenate([m[nm] for m in in_maps], axis=0)
                 for nm in in_names]
    zeros = [np.zeros((NC * a.shape[0], *a.shape[1:]), a.dtype) for a in out_avals]
    outs = fn(*concat_in, *zeros)
    res = []
    for c in range(NC):
        d = {}
        for i, nm in enumerate(out_names):
            shp = out_avals[i].shape
            d[nm] = np.asarray(outs[i]).reshape(NC, *shp)[c]
        res.append(d)
    return res


def kernel(**inputs):
    in_maps = prep_inputs(**inputs)
    res = run_in_maps(in_maps)
    total = res[0]["out"].astype(np.float32)
    for c in range(1, NC):
        total = total + res[c]["out"]
    return total.reshape(1, S, HID)
